# revision 50
# baseline (speedup 1.0000x reference)
"""Bass/Trainium2 kernel for nn_LogRatio loss, data-parallel over anchor rows on 8 cores.

Math: loss = sum_{m,j,k,l} pos[j,k] * N_m[j,l] * (A[j,k] - A[j,l] - c_m)^2
with A = log(X @ X.T + eps). Expanding the square reduces everything to
per-anchor reductions S1/S2 (pos-masked) and T1S/T1C/T2S (neg-masked),
and every mask depends on labels only through the anchor's class t_j
(24 classes). So each masked row-reduction is a matmul of A (or A^2)
against a per-class 0/1 table W[l, c]: G[j, c] = sum_l A[l, j] W[l, c],
followed by a per-row pick of column c = t_j.

Device work per core (256 anchor rows): packed bf16 input [128, 3200]
(X^T rolled so this core's anchors are columns 0:256, interleaved with
per-chunk label tables) DMA'd as four column groups so compute starts
after the first quarter lands; 16 sim matmuls (bf16), 4 quad-wide Ln
activations -> bf16 A, 4 bf16 squares, 64 bf16 G-matmuls accumulating
into one PSUM bank (single accumulation group: PSUM start=True lazily
zeroes the whole 2KB zero region, so only the first matmul starts and
only the last stops), one copy + DMA out. The O(n) epilogue (class
column selection, diagonal correction, combine, final sum) runs on the
host from the returned G tables.
"""

import numpy as np
import ml_dtypes

N, D, KK, C = 2048, 128, 4, 24
NCORES = 8
JPC = N // NCORES    # 256 anchor rows per core
NBLK = JPC // 128    # 2 blocks of 128 rows
NCH = N // 128       # 16 l-chunks
NQ = 4               # DMA/activation quads: 4 chunks each
WCOL = 72            # [Wpos 24 | Wsum 24 | Wc 24]; g2 reuses cols 0:48
GOUT = 120           # per-block G cols: [g1 72 | g2 48]
GRP = 4 * 128 + 4 * WCOL  # 800 cols per group: 4 x-chunks + their tables
PACK_COLS = NQ * GRP      # 3200
EPS = 1e-6
OMEGA = 0.1

_cache: dict = {}


def _xt_col(c):
    return (c // 4) * GRP + (c % 4) * 128


def _wt_col(c):
    return (c // 4) * GRP + 512 + (c % 4) * WCOL


def _build(repeats: int, unroll: int = 1, groups=(4, 4, 4, 4), ncuts: int = 4,
           psg_bufs: int = 2, inp_bufs: int = 5, work_bufs: int = 6):
    import concourse.bacc as bacc
    import concourse.mybir as mybir
    import concourse.tile as tile

    f32 = mybir.dt.float32
    bf16 = mybir.dt.bfloat16
    AF = mybir.ActivationFunctionType
    AL = mybir.AluOpType

    nc = bacc.Bacc("TRN2", target_bir_lowering=False, debug=False)
    pack_d = nc.dram_tensor("pack", [128, PACK_COLS], bf16, kind="ExternalInput")
    gout_d = nc.dram_tensor("gout", [128, NBLK * GOUT], f32, kind="ExternalOutput")

    with tile.TileContext(nc) as tc:
        with (
            tc.tile_pool(name="const", bufs=1) as const,
            tc.tile_pool(name="inp", bufs=inp_bufs) as inp,
            tc.tile_pool(name="work", bufs=work_bufs) as work,
            tc.tile_pool(name="psg", bufs=psg_bufs, space="PSUM") as psg,
            tc.tile_pool(name="psim", bufs=2, space="PSUM") as psim,
        ):
            epsb = const.tile([128, 1], f32, tag="epsb")
            nc.vector.memset(epsb[:], EPS)

            def body(throughput: bool = False):
                pack = inp.tile([128, PACK_COLS], bf16, tag="pack")
                step = NQ // ncuts
                for q in range(ncuts):
                    lo, hi = q * step * GRP, (q + 1) * step * GRP
                    nc.sync.dma_start(pack[:, lo:hi], pack_d[:, lo:hi])
                xj = pack[:, 0:JPC]
                gbank = psg.tile([128, 512], f32, tag="g")
                g = gbank[:, 0:NBLK * GOUT]

                c0s = [sum(groups[:i]) for i in range(len(groups))]

                def sim_mms(q, sim):
                    for k in range(groups[q]):
                        c = c0s[q] + k
                        nc.tensor.matmul(
                            sim[:, k * JPC:(k + 1) * JPC],
                            pack[:, _xt_col(c):_xt_col(c) + 128], xj,
                            start=True, stop=True,
                        )

                ngrp = len(groups)
                mw = max(groups) * JPC  # uniform tile size keeps pool rings simple
                sims = [None] * ngrp
                sims[0] = psim.tile([128, mw], f32, tag="sim", name="sim0")
                sim_mms(0, sims[0])
                for q in range(ngrp):
                    gw = groups[q] * JPC
                    a = work.tile([128, mw], bf16, tag="a", name=f"a{q}")
                    nc.scalar.activation(a[:, 0:gw], sims[q][:, 0:gw],
                                         AF.Ln, bias=epsb[:])
                    a2 = work.tile([128, mw], bf16, tag="a2", name=f"a2{q}")
                    nc.vector.tensor_mul(a2[:, 0:gw], a[:, 0:gw], a[:, 0:gw])
                    if q + 1 < ngrp:
                        sims[q + 1] = psim.tile([128, mw], f32,
                                                tag="sim", name=f"sim{q + 1}")
                        sim_mms(q + 1, sims[q + 1])
                    for k in range(groups[q]):
                        c = c0s[q] + k
                        wc = pack[:, _wt_col(c):_wt_col(c) + WCOL]
                        for b in range(NBLK):
                            blk = slice(k * JPC + b * 128, k * JPC + (b + 1) * 128)
                            nc.tensor.matmul(
                                g[:, b * GOUT:b * GOUT + 72],
                                a[:, blk], wc[:, 0:72],
                                start=(c == 0 and b == 0),
                                stop=False,
                            )
                            nc.tensor.matmul(
                                g[:, b * GOUT + 72:b * GOUT + GOUT],
                                a2[:, blk], wc[:, 0:48],
                                start=False,
                                stop=(c == NCH - 1 and b == NBLK - 1),
                            )

                gsb = work.tile([128, NBLK * GOUT], f32, tag="gsb")
                if throughput:
                    # Overlapped bodies: keep the Act engine free for the Ln
                    # chain (it is the steady-state bottleneck). Copy on DVE,
                    # out-DMA on the idle gpsimd queue — both off the Act
                    # queue and off SP (which prefetches the next body).
                    nc.vector.tensor_scalar(gsb[:], g[:], 1.0, None, AL.mult)
                    nc.gpsimd.dma_start(gout_d[:], gsb[:])
                else:
                    # Single-shot: Act copy + Act-issued DMA has the fewest
                    # cross-engine hops on the critical path.
                    nc.scalar.copy(gsb[:], g[:])
                    nc.scalar.dma_start(gout_d[:], gsb[:])

            if repeats == 1:
                body()
            else:
                assert repeats % unroll == 0
                with tc.For_i(0, repeats // unroll, 1):
                    for _ in range(unroll):
                        body(unroll > 1)

    nc.compile()
    return nc


def _host_tables(labels: np.ndarray):
    """Per-class 0/1 tables [N, 120] and per-class aggregate stats."""
    lab = np.asarray(labels).astype(np.int64)
    t = lab[:, 0]
    E = (lab[:, :, None] == np.arange(C)[None, None, :]).astype(np.float32)  # [N,4,C]
    Wpos = E[:, 0]
    W0 = 1.0 - E[:, 3]
    W1 = E[:, 3] * (1.0 - E[:, 2])
    W2 = E[:, 2] * (1.0 - E[:, 1])
    W3 = E[:, 1] * (1.0 - E[:, 0])
    cm = np.array(
        [0.1 * (np.log(OMEGA + EPS) - np.log(OMEGA ** (KK - m + 1) + EPS)) for m in range(KK)],
        dtype=np.float64,
    )
    Wsum = W0 + W1 + W2 + W3
    Wc = (cm[0] * W0 + cm[1] * W1 + cm[2] * W2 + cm[3] * W3).astype(np.float32)
    Wtbl = np.concatenate([Wpos, Wsum, Wc], axis=1)  # [N, 72]

    colsum = np.stack([W.sum(axis=0) for W in (W0, W1, W2, W3)]).astype(np.float64)
    cnt0 = Wpos.sum(axis=0).astype(np.float64)
    NnS_c = colsum.sum(axis=0)
    NnC_c = (cm[:, None] * colsum).sum(axis=0)
    NnC2_c = ((cm ** 2)[:, None] * colsum).sum(axis=0)
    return t, Wtbl, cnt0, NnS_c, NnC_c, NnC2_c, cm


def _prep_inputs(inputs: np.ndarray, labels: np.ndarray):
    X = np.asarray(inputs, dtype=np.float32)
    t, Wtbl, *_ = _host_tables(labels)
    XTb = np.ascontiguousarray(X.T).astype(ml_dtypes.bfloat16)  # [128, 2048]
    Wb = Wtbl.astype(ml_dtypes.bfloat16)                         # [2048, 120]

    in_maps = []
    for core in range(NCORES):
        j0 = core * JPC
        xt_r = np.roll(XTb, -j0, axis=1)                  # anchors at cols 0:256
        w_r = np.roll(Wb, -j0, axis=0)                    # same l-order as xt_r
        wp = w_r.reshape(NCH, 128, WCOL).transpose(1, 0, 2)  # [128, NCH, WCOL]
        pack = np.empty((128, PACK_COLS), dtype=ml_dtypes.bfloat16)
        for q in range(NQ):
            base = q * GRP
            pack[:, base:base + 512] = xt_r[:, q * 512:(q + 1) * 512]
            pack[:, base + 512:base + GRP] = (
                wp[:, 4 * q:4 * q + 4].reshape(128, 4 * WCOL)
            )
        in_maps.append({"pack": pack})
    return in_maps


def _host_epilogue(inputs, labels, gouts):
    """Combine per-core G tables [128, 240] into the scalar loss."""
    X = np.asarray(inputs, dtype=np.float32)
    t, _, cnt0, NnS_c, NnC_c, NnC2_c, _ = _host_tables(labels)

    # G[j, stat]: device row p, block b -> local anchor j = b*128 + p
    G = np.stack(gouts).reshape(NCORES, 128, NBLK, 5, C)     # [core, p, b, stat, c]
    G = G.transpose(0, 2, 1, 3, 4).reshape(N, 5, C)           # anchor-major
    sel = G[np.arange(N), :, t].astype(np.float64)            # [N, 5]
    S1g, T1S, T1C, S2g, T2S = sel.T

    # diagonal A[j,j] as the device computed it: bf16 X -> f32 dot -> ln -> bf16
    Xb = X.astype(ml_dtypes.bfloat16).astype(np.float32)
    simjj = (Xb * Xb).sum(axis=1)
    ajj = np.log(simjj + EPS).astype(ml_dtypes.bfloat16)
    dA = ajj.astype(np.float64)
    dA2 = (ajj * ajj).astype(ml_dtypes.bfloat16).astype(np.float64)

    S1 = S1g - dA
    S2 = S2g - dA2
    Pn = cnt0[t] - 1.0
    NnS, NnC, NnC2 = NnS_c[t], NnC_c[t], NnC2_c[t]
    L = NnS * S2 - 2.0 * S1 * (NnC + T1S) + Pn * (NnC2 + 2.0 * T1C + T2S)
    return np.float32(L.sum())


def _get_nc(repeats: int = 1, unroll: int = 1, ncuts: int = 4):
    key = ("nc", repeats, unroll, ncuts)
    if key not in _cache:
        _cache[key] = _build(repeats, unroll, ncuts=ncuts)
    return _cache[key]


def run_on_device(inputs, labels, repeats: int = 1):
    from concourse.bass_utils import run_bass_kernel_spmd

    nc = _get_nc(repeats)
    in_maps = _prep_inputs(inputs, labels)
    res = run_bass_kernel_spmd(nc, in_maps, list(range(NCORES)))
    gouts = [res.results[i]["gout"] for i in range(NCORES)]
    return _host_epilogue(inputs, labels, gouts)


def kernel(inputs, labels):
    total = run_on_device(inputs, labels, repeats=1)
    return (total, 0, 0, 0)
